# revision 39
# baseline (speedup 1.0000x reference)
"""Trainium2 Bass kernel for nn_BDH_39127152067244 (dense_transformer).

Sharding: 8 cores = (b, h) pairs — b = core // 4, h = core % 4. Each core
computes its head's share of every layer; the only cross-core communication
is AllReduce of the per-head yMLP partial [T, D] once per layer
(replica groups {0..3} and {4..7}).

v2 layer pipeline (vs v1):
  - yMLP partial is split over the sparse dim N into two halves (P1 =
    n-tiles 0..31, P2 = 32..63). AllReduce(P1) launches halfway through
    the fused E/F phase and hides under ~27us of remaining matmuls;
    AllReduce(P2) is split into two chunk-pair collectives so the
    LN + transpose chain for t-chunks (0,1) overlaps the second one.
  - F(g) is staggered one group behind E(g) so the relu+gate chain for
    XY[g] never stalls the PE.
  - The first A group of the next layer is t-half split (and borrows the
    C-phase PSUM banks for 4 concurrent quads) so the PE restarts as soon
    as half of the new x is normalized.
  - rope's partner multiply runs on GpSimd, one relu quad per A group on
    Vector, the rest on Scalar — balancing the three elementwise engines
    against the (GPIO-throttled) PE streaming rate.
  - LayerNorm rstd uses a single Rsqrt activation.

Layout tricks (unchanged from v1):
  - The N axis (8192) is deinterleaved on the host (even n first, odd n
    second), applied consistently to encoder / encoder_v / decoder rows and
    the rope tables. Rope's interleaved pair-swap then becomes a clean
    half-offset of whole 128-partition tiles with a sign folded into the
    sin table.
  - x_sparse is computed directly in transposed [N, T] layout; the masked
    Gram matrix is computed directly as the yKV matmul's lhsT in [s, t]
    layout (symmetry), skipping fully-masked tiles.
  - All matmuls run in bf16 with f32 PSUM accumulation; LayerNorms and the
    residual stream stay f32.
"""

import math
import sys
from contextlib import ExitStack

import numpy as np
import ml_dtypes

sys.path.insert(0, "/opt/trn_rl_repo")

import concourse.bass as bass  # noqa: E402
import concourse.bacc as bacc  # noqa: E402
import concourse.mybir as mybir  # noqa: E402
import concourse.tile as tile  # noqa: E402
from concourse.bass import ds  # noqa: E402
from concourse.bass_utils import run_bass_kernel_spmd  # noqa: E402
from concourse.masks import make_identity  # noqa: E402

BF16 = ml_dtypes.bfloat16
BF = mybir.dt.bfloat16
FP32 = mybir.dt.float32
AF = mybir.ActivationFunctionType
ALU = mybir.AluOpType

# Problem constants (hardcoded per the harness contract).
N_LAYER = 6
D = 256
NH = 4
N = 8192
HALF = N // 2
VOCAB = 256
B, T = 2, 512
THETA = 2.0**16
EPS = 1e-5

P = 128          # partitions
HT = T // 2      # 256: t-half for the split A groups
NT = N // P      # 64 n-tiles
G4 = 4           # n-tiles per rope/qx group
NG = NT // G4    # 16 groups
VG = 8           # n-tiles per V tile
NVG = NT // VG   # 8 V tiles
TC = T // P      # 4 t-chunks
DT = D // P      # 2 d-tiles
N_CORES = 8
GROUPS = [[0, 1, 2, 3], [4, 5, 6, 7]]

_CACHE: dict = {}


def _build_bass():
    nc = bacc.Bacc("TRN2", num_devices=N_CORES)

    x0_d = nc.dram_tensor("x0", [P, TC, D], FP32, kind="ExternalInput")
    x0bf_d = nc.dram_tensor("x0bf", [P, TC, D], BF, kind="ExternalInput")
    x0T_d = nc.dram_tensor("x0T", [P, DT, T], BF, kind="ExternalInput")
    enc_d = nc.dram_tensor("enc", [DT, P, NT, P], BF, kind="ExternalInput")
    encv_d = nc.dram_tensor("encv", [DT, P, NT, P], BF, kind="ExternalInput")
    dec_d = nc.dram_tensor("dec", [P, NT, D], BF, kind="ExternalInput")
    cos_d = nc.dram_tensor("cosb", [P, NT, T], BF, kind="ExternalInput")
    sin_d = nc.dram_tensor("sinb", [P, NT, T], BF, kind="ExternalInput")
    mask_d = nc.dram_tensor("maskb", [P, TC, T], BF, kind="ExternalInput")
    lm_d = nc.dram_tensor("lm", [P, DT, VOCAB], BF, kind="ExternalInput")
    out_d = nc.dram_tensor("logits", [P, TC, VOCAB], FP32, kind="ExternalOutput")

    with tile.TileContext(nc) as tc, ExitStack() as ctx:
        sb = ctx.enter_context(tc.tile_pool(name="sb", bufs=1))
        vpool = ctx.enter_context(tc.tile_pool(name="vpool", bufs=NVG))
        qxpool = ctx.enter_context(tc.tile_pool(name="qxpool", bufs=8))
        wpool = ctx.enter_context(tc.tile_pool(name="wpool", bufs=2))
        tabpool = ctx.enter_context(tc.tile_pool(name="tabpool", bufs=2))
        roppool = ctx.enter_context(tc.tile_pool(name="roppool", bufs=2))
        mixpool = ctx.enter_context(tc.tile_pool(name="mixpool", bufs=2))
        # Layer-spaced tiles (dead long before their next-layer reuse) —
        # single-buffered to pay for the deeper qx ring.
        mix1pool = ctx.enter_context(tc.tile_pool(name="mix1pool", bufs=1))
        statpool = ctx.enter_context(tc.tile_pool(name="statpool", bufs=8))
        xpool = ctx.enter_context(tc.tile_pool(name="xpool", bufs=2))
        apsum = ctx.enter_context(tc.tile_pool(name="apsum", bufs=2, space="PSUM"))
        cpsum = ctx.enter_context(tc.tile_pool(name="cpsum", bufs=1, space="PSUM"))
        drm = ctx.enter_context(tc.tile_pool(name="drm", bufs=2, space="DRAM"))

        ident = sb.tile([P, P], BF, name="ident")
        make_identity(nc, ident)
        epst = sb.tile([P, 1], FP32, name="epst")
        nc.vector.memset(epst, EPS)
        # x_T first: layer 0's first matmuls wait only on it (+ enc tiles).
        x_T = xpool.tile([P, DT, T], BF, tag="xT", name="x_T0")
        nc.sync.dma_start(out=x_T, in_=x0T_d[:])
        x_bf = xpool.tile([P, TC, D], BF, tag="xbf", name="x_bf0")
        nc.sync.dma_start(out=x_bf, in_=x0bf_d[:])
        x_f = xpool.tile([P, TC, D], FP32, tag="xf", name="x_f0")
        nc.sync.dma_start(out=x_f, in_=x0_d[:])
        maskt = sb.tile([P, TC, T], BF, name="maskt")
        nc.sync.dma_start(out=maskt, in_=mask_d[:])
        lmt = sb.tile([P, DT, VOCAB], BF, name="lmt")
        nc.sync.dma_start(out=lmt, in_=lm_d[:])

        def layer_norm_stats(src_ap, name):
            """Returns (mv, rstd) where mv[:,0:1]=mean, rstd=1/sqrt(var+eps)."""
            stats = statpool.tile([P, 6], FP32, tag="bst", name=f"st_{name}")
            nc.vector.bn_stats(out=stats, in_=src_ap)
            mv = statpool.tile([P, 2], FP32, tag="bmv", name=f"mv_{name}")
            nc.vector.bn_aggr(out=mv, in_=stats)
            rstd = statpool.tile([P, 1], FP32, tag="brs", name=f"rs_{name}")
            nc.scalar.activation(out=rstd, in_=mv[:, 1:2], func=AF.Sqrt, bias=epst)
            nc.vector.reciprocal(rstd, rstd)
            return mv, rstd

        def layer_norm_stats_batch(srcs, name):
            """Stage-ordered LN stats for several chunks: all bn_stats+aggr
            first, then the ACT sqrts, then the DVE recips — avoids DVE
            head-of-line blocking on the ACT round-trip."""
            mvs, rstds = [], []
            for i, src in enumerate(srcs):
                stats = statpool.tile([P, 6], FP32, tag="bst", name=f"st_{name}{i}")
                nc.vector.bn_stats(out=stats, in_=src)
                mv = statpool.tile([P, 2], FP32, tag="bmv", name=f"mv_{name}{i}")
                nc.vector.bn_aggr(out=mv, in_=stats)
                mvs.append(mv)
            for i, mv in enumerate(mvs):
                rstd = statpool.tile([P, 1], FP32, tag="brs", name=f"rs_{name}{i}")
                nc.scalar.activation(
                    out=rstd, in_=mv[:, 1:2], func=AF.Sqrt, bias=epst
                )
                rstds.append(rstd)
            for rstd in rstds:
                nc.vector.reciprocal(rstd, rstd)
            return list(zip(mvs, rstds))

        def emit_layer(l, x_f, x_bf, x_T):
            # ---------------- step A: V^T = relu(enc^T @ x^T), [N, T] ------
            V = [None] * NVG

            def emit_A(vg, split=False):
                encg = wpool.tile([P, DT, VG, P], BF, tag="w", name=f"enc{l}_{vg}")
                nc.sync.dma_start(
                    out=encg,
                    in_=enc_d[:, :, ds(vg * VG, VG), :].rearrange(
                        "dt p nt n -> p dt nt n"
                    ),
                )
                vt = vpool.tile([P, VG, T], BF, tag="v", name=f"v{l}_{vg}")
                V[vg] = vt
                if split:
                    # Borrow the (currently idle) C-phase PSUM banks for two
                    # extra quads so all four quads' first-half matmuls can
                    # run before the second t-half of x_T is ready.
                    aq = cpsum.tile([P, TC, T], FP32, tag="mm", name=f"aq{l}")
                    q01 = [aq[:, 0:2, :], aq[:, 2:4, :]]
                    q23 = [
                        apsum.tile([P, 2, T], FP32, tag="quad", name=f"as{l}_{q}")
                        for q in range(2)
                    ]
                    quads = q01 + q23
                    for h in range(2):
                        for q in range(4):
                            for i in range(2):
                                for dt_ in range(DT):
                                    nc.tensor.matmul(
                                        quads[q][:, i, ds(h * HT, HT)],
                                        lhsT=encg[:, dt_, q * 2 + i, :],
                                        rhs=x_T[:, dt_, ds(h * HT, HT)],
                                        start=(dt_ == 0),
                                        stop=(dt_ == DT - 1),
                                    )
                    for q in range(4):
                        nc.scalar.activation(
                            out=vt[:, ds(q * 2, 2), :], in_=quads[q],
                            func=AF.Relu,
                        )
                    return
                for q in range(VG // 2):
                    ps = apsum.tile(
                        [P, 2, T], FP32, tag="quad", name=f"aps{l}_{vg}_{q}"
                    )
                    for i in range(2):
                        for dt_ in range(DT):
                            nc.tensor.matmul(
                                ps[:, i, :],
                                lhsT=encg[:, dt_, q * 2 + i, :],
                                rhs=x_T[:, dt_, :],
                                start=(dt_ == 0),
                                stop=(dt_ == DT - 1),
                            )
                    nc.scalar.activation(
                        out=vt[:, ds(q * 2, 2), :], in_=ps, func=AF.Relu
                    )

            # ---------------- rope: QR = V*cos + Vpartner*sin' -------------
            QR = [None] * NG

            def emit_rope(g):
                cosg = tabpool.tile([P, G4, T], BF, tag="cos", name=f"cos{l}_{g}")
                nc.sync.dma_start(out=cosg, in_=cos_d[:, ds(g * G4, G4), :])
                sing = tabpool.tile([P, G4, T], BF, tag="sin", name=f"sin{l}_{g}")
                nc.sync.dma_start(out=sing, in_=sin_d[:, ds(g * G4, G4), :])
                qr = qxpool.tile([P, G4, T], BF, tag="qx", name=f"qr{l}_{g}")
                QR[g] = qr
                pg = roppool.tile([P, G4, T], BF, tag="rp", name=f"rp{l}_{g}")
                p2 = roppool.tile([P, G4, T], BF, tag="rp2", name=f"rq{l}_{g}")
                vg_, off = divmod(g * G4, VG)
                pvg_, poff = divmod((g ^ (NG // 2)) * G4, VG)
                nc.vector.tensor_mul(pg, V[vg_][:, ds(off, G4), :], cosg)
                nc.vector.tensor_mul(p2, V[pvg_][:, ds(poff, G4), :], sing)
                nc.vector.tensor_add(qr, pg, p2)

            for pair in range(NVG // 2):
                emit_A(pair, split=(pair == 0))
                emit_A(pair + NVG // 2)
                emit_rope(pair * 2)
                emit_rope(pair * 2 + 1)
            for g in range(NG // 2, NG):
                emit_rope(g)

            # ---------------- step C: masked Gram in [s, t] ----------------
            gps = cpsum.tile([P, TC, T], FP32, tag="mm", name=f"gps{l}")
            for k in range(NT):
                g, i = divmod(k, G4)
                for j in range(TC):
                    nc.tensor.matmul(
                        gps[:, j, : T - j * P],
                        lhsT=QR[g][:, i, ds(j * P, P)],
                        rhs=QR[g][:, i, ds(j * P, T - j * P)],
                        start=(k == 0),
                        stop=(k == NT - 1),
                    )
            # PSUM -> SBUF cast: only the diagonal 128-blocks need the strict
            # mask (off-diagonal blocks are all-ones) — mask-mul them on
            # Vector while Scalar copies the off-diagonal spans in parallel.
            st = mix1pool.tile([P, TC, T], BF, tag="st", name=f"st{l}")
            for j in range(TC):
                nc.vector.tensor_mul(
                    st[:, j, ds(j * P, P)],
                    gps[:, j, :P],
                    maskt[:, j, ds(j * P, P)],
                )
                if j < TC - 1:
                    nc.scalar.copy(
                        out=st[:, j, ds((j + 1) * P, T - (j + 1) * P)],
                        in_=gps[:, j, ds(P, T - (j + 1) * P)],
                    )

            # ---------------- step D: yKV = M^T @ x, then LN ---------------
            dps = cpsum.tile([P, TC, T], FP32, tag="mm", name=f"dps{l}")
            for jp in range(TC):
                for i in range(jp + 1):
                    nc.tensor.matmul(
                        dps[:, jp, :D],
                        lhsT=st[:, i, ds(jp * P, P)],
                        rhs=x_bf[:, i, :],
                        start=(i == 0),
                        stop=(i == jp),
                    )
            yln = mix1pool.tile([P, TC, D], BF, tag="yln", name=f"yln{l}")
            dstats = layer_norm_stats_batch(
                [dps[:, jp, :D] for jp in range(TC)], f"d{l}"
            )
            for jp in range(TC):
                mv, rstd = dstats[jp]
                nc.vector.tensor_scalar(
                    out=yln[:, jp, :],
                    in0=dps[:, jp, :D],
                    scalar1=mv[:, 0:1],
                    scalar2=rstd,
                    op0=ALU.subtract,
                    op1=ALU.mult,
                )
            ylnT = mix1pool.tile([P, DT, T], BF, tag="ylnT", name=f"ylnT{l}")
            for hv in range(2):
                tp = apsum.tile(
                    [P, DT, 2, P], BF, tag="quad", name=f"ytp{l}_{hv}"
                )
                for dt_ in range(DT):
                    for ji, jp in enumerate(range(hv * 2, hv * 2 + 2)):
                        nc.tensor.transpose(
                            tp[:, dt_, ji, :], yln[:, jp, ds(dt_ * P, P)], ident
                        )
                for dt_ in range(DT):
                    nc.scalar.copy(
                        out=ylnT[:, dt_, ds(hv * HT, HT)].rearrange(
                            "p (a b) -> p a b", a=2
                        ),
                        in_=tp[:, dt_],
                    )

            # ---------------- steps E+F fused: gated y_sparse + yMLP -------
            # F(g) staggered one group behind E(g); yMLP partial split over
            # n: P1 = groups 0..7, P2 = groups 8..15, accumulated into the
            # two column-halves of one C-phase PSUM tile.
            fpst = cpsum.tile([P, TC, T], FP32, tag="mm", name=f"fpst{l}")
            fhalf = [fpst[:, :, 0:D], fpst[:, :, ds(D, D)]]
            XY = [None] * NG
            EV = [None] * NVG

            def emit_E(g, split=False):
                vg, half = divmod(g, 2)
                if half == 0:
                    evg = wpool.tile(
                        [P, DT, VG, P], BF, tag="w", name=f"ev{l}_{vg}"
                    )
                    EV[vg] = evg
                    nc.sync.dma_start(
                        out=evg,
                        in_=encv_d[:, :, ds(vg * VG, VG), :].rearrange(
                            "dt p nt n -> p dt nt n"
                        ),
                    )
                evg = EV[vg]
                xy = qxpool.tile([P, G4, T], BF, tag="qx", name=f"xy{l}_{g}")
                XY[g] = xy
                for q in range(2):
                    ps = apsum.tile(
                        [P, 2, T], FP32, tag="quad", name=f"eps{l}_{g}_{q}"
                    )
                    if split:
                        for h in range(2):
                            for i in range(2):
                                nt_ = half * G4 + q * 2 + i
                                for dt_ in range(DT):
                                    nc.tensor.matmul(
                                        ps[:, i, ds(h * HT, HT)],
                                        lhsT=evg[:, dt_, nt_, :],
                                        rhs=ylnT[:, dt_, ds(h * HT, HT)],
                                        start=(dt_ == 0),
                                        stop=(dt_ == DT - 1),
                                    )
                    else:
                        for i in range(2):
                            nt_ = half * G4 + q * 2 + i
                            for dt_ in range(DT):
                                nc.tensor.matmul(
                                    ps[:, i, :],
                                    lhsT=evg[:, dt_, nt_, :],
                                    rhs=ylnT[:, dt_, :],
                                    start=(dt_ == 0),
                                    stop=(dt_ == DT - 1),
                                )
                    ys = roppool.tile(
                        [P, 2, T], BF, tag="rp2", name=f"ys{l}_{g}_{q}"
                    )
                    nc.scalar.activation(out=ys, in_=ps, func=AF.Relu)
                    nc.vector.tensor_mul(
                        xy[:, ds(q * 2, 2), :],
                        ys,
                        V[vg][:, ds(half * G4 + q * 2, 2), :],
                    )

            def emit_F(g, m_outer=False):
                decg = wpool.tile([P, G4, D], BF, tag="dec", name=f"dec{l}_{g}")
                nc.sync.dma_start(out=decg, in_=dec_d[:, ds(g * G4, G4), :])
                tgt = fhalf[g // 8]
                loop = (
                    [(i, m) for m in range(TC) for i in range(G4)]
                    if m_outer
                    else [(i, m) for i in range(G4) for m in range(TC)]
                )
                for i, m in loop:
                    kk = (g % 8) * G4 + i
                    nc.tensor.matmul(
                        tgt[:, m, :],
                        lhsT=XY[g][:, i, ds(m * P, P)],
                        rhs=decg[:, i, :],
                        start=(kk == 0),
                        stop=(kk == NT // 2 - 1),
                    )

            ccin1 = drm.tile([P, TC, D], BF, tag="ccin1", name=f"ccin1_{l}")
            ccout1 = drm.tile([P, TC, D], BF, tag="ccout1", name=f"ccout1_{l}")
            ymr1 = mixpool.tile([P, TC, D], BF, tag="ymr1", name=f"ymr1_{l}")
            for g in range(NG):
                emit_E(g, split=(g < 2))
                if g >= 2:
                    emit_F(g - 2)
                if g == 9:
                    # P1 complete: launch AllReduce #1 under the remaining
                    # E/F matmul stream.
                    ym1 = mixpool.tile([P, TC, D], BF, tag="ym1", name=f"ym1_{l}")
                    nc.scalar.copy(out=ym1, in_=fhalf[0])
                    nc.sync.dma_start(out=ccin1[:], in_=ym1)
                    nc.gpsimd.collective_compute(
                        "AllReduce",
                        ALU.add,
                        replica_groups=GROUPS,
                        ins=[ccin1[:]],
                        outs=[ccout1[:]],
                    )
                    nc.sync.dma_start(out=ymr1, in_=ccout1[:])
            emit_F(NG - 2)
            emit_F(NG - 1, m_outer=True)

            # P2 complete: two chunk-pair AllReduces (copies on Scalar — it
            # wakes instantly after the accumulation stop, unlike Vector).
            # On the last layer there is no next-layer work to overlap the
            # second collective, so one merged AllReduce (which finishes
            # earlier than the serialized pair) shortens the tail instead.
            last = l == N_LAYER - 1
            if last:
                ym2 = mixpool.tile([P, TC, D], BF, tag="ym1", name=f"ym2_{l}")
                nc.scalar.copy(out=ym2, in_=fhalf[1])
                cc_in = drm.tile([P, TC, D], BF, tag="ccin1", name=f"ccin2_{l}")
                cc_out = drm.tile([P, TC, D], BF, tag="ccout1", name=f"ccout2_{l}")
                nc.sync.dma_start(out=cc_in[:], in_=ym2)
                nc.gpsimd.collective_compute(
                    "AllReduce",
                    ALU.add,
                    replica_groups=GROUPS,
                    ins=[cc_in[:]],
                    outs=[cc_out[:]],
                )
                ymr2m = mixpool.tile([P, TC, D], BF, tag="ymr1", name=f"ymr2_{l}")
                nc.sync.dma_start(out=ymr2m, in_=cc_out[:])
                ymr2 = [ymr2m[:, 0:2, :], ymr2m[:, 2:4, :]]
            else:
                ymr2 = [None, None]
            for hv in range(2 if not last else 0):
                ym2 = mixpool.tile(
                    [P, 2, D], BF, tag=f"ym2{hv}", name=f"ym2_{l}_{hv}"
                )
                nc.scalar.copy(out=ym2, in_=fhalf[1][:, ds(hv * 2, 2), :])
                cc_in = drm.tile(
                    [P, 2, D], BF, tag=f"ccin2{hv}", name=f"ccin2_{l}_{hv}"
                )
                cc_out = drm.tile(
                    [P, 2, D], BF, tag=f"ccout2{hv}", name=f"ccout2_{l}_{hv}"
                )
                nc.sync.dma_start(out=cc_in[:], in_=ym2)
                nc.gpsimd.collective_compute(
                    "AllReduce",
                    ALU.add,
                    replica_groups=GROUPS,
                    ins=[cc_in[:]],
                    outs=[cc_out[:]],
                )
                ymr2[hv] = mixpool.tile(
                    [P, 2, D], BF, tag=f"ymr2{hv}", name=f"ymr2_{l}_{hv}"
                )
                nc.sync.dma_start(out=ymr2[hv], in_=cc_out[:])

            # ---------------- x = LN(x + LN(yMLP)), per chunk-pair ---------
            x_f_new = xpool.tile([P, TC, D], FP32, tag="xf", name=f"x_f{l + 1}")
            x_bf_new = xpool.tile([P, TC, D], BF, tag="xbf", name=f"x_bf{l + 1}")
            x_T_new = xpool.tile([P, DT, T], BF, tag="xT", name=f"x_T{l + 1}")
            xmid = mixpool.tile([P, TC, D], FP32, tag="xmid", name=f"xm{l}")
            last = l == N_LAYER - 1
            if last:
                lps = cpsum.tile([P, TC, T], FP32, tag="mm", name="lps")
                lout = mixpool.tile([P, TC, VOCAB], FP32, tag="lout", name="lout")
            ysum = mixpool.tile([P, TC, D], BF, tag="ysum", name=f"ys{l}")
            for hv in range(2):
                jps = list(range(hv * 2, hv * 2 + 2))
                nc.vector.tensor_add(
                    ysum[:, ds(hv * 2, 2), :],
                    ymr1[:, ds(hv * 2, 2), :],
                    ymr2[hv],
                )
                ystats = layer_norm_stats_batch(
                    [ysum[:, jp, :] for jp in jps], f"y{l}_{hv}"
                )
                for ji, jp in enumerate(jps):
                    nc.vector.scalar_tensor_tensor(
                        out=xmid[:, jp, :],
                        in0=ysum[:, jp, :],
                        scalar=ystats[ji][1],
                        in1=x_f[:, jp, :],
                        op0=ALU.mult,
                        op1=ALU.add,
                    )
                xstats = layer_norm_stats_batch(
                    [xmid[:, jp, :] for jp in jps], f"x{l}_{hv}"
                )
                for ji, jp in enumerate(jps):
                    mv2, r2 = xstats[ji]
                    nc.vector.tensor_scalar(
                        out=x_bf_new[:, jp, :],
                        in0=xmid[:, jp, :],
                        scalar1=mv2[:, 0:1],
                        scalar2=r2,
                        op0=ALU.subtract,
                        op1=ALU.mult,
                    )
                # One PSUM tile per chunk-pair for all four transposes —
                # halves the quad-ring pressure so the next layer's split-A
                # quads are not WAR-blocked behind the hv=1 transposes.
                tp = apsum.tile(
                    [P, DT, 2, P], BF, tag="quad", name=f"xtp{l}_{hv}"
                )
                for dt_ in range(DT):
                    for ji, jp in enumerate(jps):
                        nc.tensor.transpose(
                            tp[:, dt_, ji, :],
                            x_bf_new[:, jp, ds(dt_ * P, P)],
                            ident,
                        )
                for dt_ in range(DT):
                    nc.scalar.copy(
                        out=x_T_new[:, dt_, ds(hv * HT, HT)].rearrange(
                            "p (a b) -> p a b", a=2
                        ),
                        in_=tp[:, dt_],
                    )
                if last:
                    # lm head folded into the final boundary, per chunk-pair.
                    for jp in jps:
                        for dt_ in range(DT):
                            nc.tensor.matmul(
                                lps[:, jp, :VOCAB],
                                lhsT=x_T_new[:, dt_, ds(jp * P, P)],
                                rhs=lmt[:, dt_, :],
                                start=(dt_ == 0),
                                stop=(dt_ == DT - 1),
                            )
                    nc.scalar.copy(
                        out=lout[:, ds(hv * 2, 2), :],
                        in_=lps[:, ds(hv * 2, 2), :VOCAB],
                    )
                    nc.sync.dma_start(
                        out=out_d[:, ds(hv * 2, 2), :],
                        in_=lout[:, ds(hv * 2, 2), :],
                    )
                else:
                    for ji, jp in enumerate(jps):
                        mv2, r2 = xstats[ji]
                        nc.vector.tensor_scalar(
                            out=x_f_new[:, jp, :],
                            in0=xmid[:, jp, :],
                            scalar1=mv2[:, 0:1],
                            scalar2=r2,
                            op0=ALU.subtract,
                            op1=ALU.mult,
                        )
            return x_f_new, x_bf_new, x_T_new

        for l in range(N_LAYER):
            x_f, x_bf, x_T = emit_layer(l, x_f, x_bf, x_T)

    if not nc.is_finalized():
        nc.finalize()
    return nc


def _ln_np(x):
    m = x.mean(-1, keepdims=True)
    v = ((x - m) ** 2).mean(-1, keepdims=True)
    return (x - m) / np.sqrt(v + EPS)


def _make_tables():
    t = np.arange(N, dtype=np.float32)
    q = np.floor(t / 2.0) * 2.0
    freqs = (1.0 / (THETA ** (q / N)) / (2.0 * np.float32(math.pi))).astype(
        np.float32
    )
    phases = np.arange(T, dtype=np.float32)[:, None] * freqs[None, :]
    ph = np.float32(np.float32(phases % 1.0) * np.float32(2.0 * math.pi))
    return np.cos(ph).astype(np.float32), np.sin(ph).astype(np.float32)


def _prep_inputs(idx, embed_w, encoder, encoder_v, decoder, lm_head):
    perm = np.concatenate([np.arange(HALF) * 2, np.arange(HALF) * 2 + 1])

    cos, sin = _make_tables()
    cosp = cos[:, perm]
    sinp = sin[:, perm].copy()
    sinp[:, :HALF] *= -1.0
    # [P, NT, T]: (p, nt, t) -> table[t, nt*P + p]
    cos_h = np.ascontiguousarray(
        cosp.T.reshape(NT, P, T).transpose(1, 0, 2)
    ).astype(BF16)
    sin_h = np.ascontiguousarray(
        sinp.T.reshape(NT, P, T).transpose(1, 0, 2)
    ).astype(BF16)

    mask_h = np.zeros((P, TC, T), np.float32)
    t_idx = np.arange(T)
    for j in range(TC):
        for p in range(P):
            mask_h[p, j] = (t_idx > (j * P + p)).astype(np.float32)
    mask_h = mask_h.astype(BF16)

    lm_h = np.ascontiguousarray(
        lm_head.reshape(DT, P, VOCAB).transpose(1, 0, 2)
    ).astype(BF16)

    x0 = _ln_np(embed_w[idx].astype(np.float32))  # (B, T, D)

    dec3 = decoder.reshape(NH, N, D)

    per_core = []
    for core in range(N_CORES):
        b, h = divmod(core, NH)
        enc_p = encoder[h][:, perm]  # (D, N)
        encv_p = encoder_v[h][:, perm]
        dec_p = dec3[h][perm, :]  # (N, D)

        enc_h = enc_p.reshape(DT, P, NT, P).astype(BF16)
        encv_h = encv_p.reshape(DT, P, NT, P).astype(BF16)
        dec_h = np.ascontiguousarray(
            dec_p.reshape(NT, P, D).transpose(1, 0, 2)
        ).astype(BF16)

        xb = x0[b]  # (T, D) f32
        x0_c = np.ascontiguousarray(
            xb.reshape(TC, P, D).transpose(1, 0, 2)
        ).astype(np.float32)
        x0bf_c = x0_c.astype(BF16)
        x0T_c = np.ascontiguousarray(
            xb.T.reshape(DT, P, T).transpose(1, 0, 2)
        ).astype(BF16)

        per_core.append(
            {
                "x0": x0_c,
                "x0bf": x0bf_c,
                "x0T": x0T_c,
                "enc": enc_h,
                "encv": encv_h,
                "dec": dec_h,
                "cosb": cos_h,
                "sinb": sin_h,
                "maskb": mask_h,
                "lm": lm_h,
            }
        )
    return per_core


def _get_nc():
    if "nc" not in _CACHE:
        _CACHE["nc"] = _build_bass()
    return _CACHE["nc"]


def kernel(idx, embed_w, encoder, encoder_v, decoder, lm_head, **extra):
    idx = np.asarray(idx)
    embed_w = np.asarray(embed_w, dtype=np.float32)
    encoder = np.asarray(encoder, dtype=np.float32)
    encoder_v = np.asarray(encoder_v, dtype=np.float32)
    decoder = np.asarray(decoder, dtype=np.float32)
    lm_head = np.asarray(lm_head, dtype=np.float32)

    nc = _get_nc()
    in_maps = _prep_inputs(idx, embed_w, encoder, encoder_v, decoder, lm_head)
    res = run_bass_kernel_spmd(nc, in_maps, core_ids=list(range(N_CORES)))
    _CACHE["last_results"] = res

    out = np.zeros((B, T, VOCAB), np.float32)
    for b in range(B):
        lg = res.results[b * NH]["logits"]  # [P, TC, VOCAB]
        out[b] = lg.transpose(1, 0, 2).reshape(T, VOCAB)
    return out


if __name__ == "__main__":
    rng = np.random.default_rng(0)
    ins = {
        "idx": rng.integers(0, VOCAB, (B, T)).astype(np.int32),
        "embed_w": (0.02 * rng.standard_normal((VOCAB, D))).astype(np.float32),
        "encoder": (0.02 * rng.standard_normal((NH, D, N))).astype(np.float32),
        "encoder_v": (0.02 * rng.standard_normal((NH, D, N))).astype(np.float32),
        "decoder": (0.02 * rng.standard_normal((NH * N, D))).astype(np.float32),
        "lm_head": (0.02 * rng.standard_normal((D, VOCAB))).astype(np.float32),
    }
    out = kernel(**ins)
    print("out", out.shape, out.dtype, float(np.abs(out).max()))


# revision 45
# speedup vs baseline: 1.0350x; 1.0350x over previous
"""Trainium2 Bass kernel for nn_BDH_39127152067244 (dense_transformer).

Sharding: 8 cores = (b, h) pairs — b = core // 4, h = core % 4. Each core
computes its head's share of every layer; the only cross-core communication
is AllReduce of the per-head yMLP partial [T, D] once per layer
(replica groups {0..3} and {4..7}).

v2 layer pipeline (vs v1):
  - yMLP partial is split over the sparse dim N into two halves (P1 =
    n-tiles 0..31, P2 = 32..63). AllReduce(P1) launches halfway through
    the fused E/F phase and hides under ~27us of remaining matmuls;
    AllReduce(P2) is split into two chunk-pair collectives so the
    LN + transpose chain for t-chunks (0,1) overlaps the second one.
  - F(g) is staggered one group behind E(g) so the relu+gate chain for
    XY[g] never stalls the PE.
  - The first A group of the next layer is t-half split (and borrows the
    C-phase PSUM banks for 4 concurrent quads) so the PE restarts as soon
    as half of the new x is normalized.
  - rope's partner multiply runs on GpSimd, one relu quad per A group on
    Vector, the rest on Scalar — balancing the three elementwise engines
    against the (GPIO-throttled) PE streaming rate.
  - LayerNorm rstd uses a single Rsqrt activation.

Layout tricks (unchanged from v1):
  - The N axis (8192) is deinterleaved on the host (even n first, odd n
    second), applied consistently to encoder / encoder_v / decoder rows and
    the rope tables. Rope's interleaved pair-swap then becomes a clean
    half-offset of whole 128-partition tiles with a sign folded into the
    sin table.
  - x_sparse is computed directly in transposed [N, T] layout; the masked
    Gram matrix is computed directly as the yKV matmul's lhsT in [s, t]
    layout (symmetry), skipping fully-masked tiles.
  - All matmuls run in bf16 with f32 PSUM accumulation; LayerNorms and the
    residual stream stay f32.
"""

import math
import sys
from contextlib import ExitStack

import numpy as np
import ml_dtypes

sys.path.insert(0, "/opt/trn_rl_repo")

import concourse.bass as bass  # noqa: E402
import concourse.bacc as bacc  # noqa: E402
import concourse.mybir as mybir  # noqa: E402
import concourse.tile as tile  # noqa: E402
from concourse.bass import ds  # noqa: E402
from concourse.bass_utils import run_bass_kernel_spmd  # noqa: E402
from concourse.masks import make_identity  # noqa: E402

BF16 = ml_dtypes.bfloat16
BF = mybir.dt.bfloat16
FP32 = mybir.dt.float32
AF = mybir.ActivationFunctionType
ALU = mybir.AluOpType

# Problem constants (hardcoded per the harness contract).
N_LAYER = 6
D = 256
NH = 4
N = 8192
HALF = N // 2
VOCAB = 256
B, T = 2, 512
THETA = 2.0**16
EPS = 1e-5

P = 128          # partitions
HT = T // 2      # 256: t-half for the split A groups
NT = N // P      # 64 n-tiles
G4 = 4           # n-tiles per rope/qx group
NG = NT // G4    # 16 groups
VG = 8           # n-tiles per V tile
NVG = NT // VG   # 8 V tiles
TC = T // P      # 4 t-chunks
DT = D // P      # 2 d-tiles
N_CORES = 8
GROUPS = [[0, 1, 2, 3], [4, 5, 6, 7]]

_CACHE: dict = {}


def _build_bass():
    nc = bacc.Bacc("TRN2", num_devices=N_CORES)

    x0_d = nc.dram_tensor("x0", [P, TC, D], FP32, kind="ExternalInput")
    x0bf_d = nc.dram_tensor("x0bf", [P, TC, D], BF, kind="ExternalInput")
    x0T_d = nc.dram_tensor("x0T", [P, DT, T], BF, kind="ExternalInput")
    enc_d = nc.dram_tensor("enc", [DT, P, NT, P], BF, kind="ExternalInput")
    encv_d = nc.dram_tensor("encv", [DT, P, NT, P], BF, kind="ExternalInput")
    dec_d = nc.dram_tensor("dec", [P, NT, D], BF, kind="ExternalInput")
    cos_d = nc.dram_tensor("cosb", [P, NT, T], BF, kind="ExternalInput")
    sin_d = nc.dram_tensor("sinb", [P, NT, T], BF, kind="ExternalInput")
    mask_d = nc.dram_tensor("maskb", [P, TC, T], BF, kind="ExternalInput")
    lm_d = nc.dram_tensor("lm", [P, DT, VOCAB], BF, kind="ExternalInput")
    out_d = nc.dram_tensor("logits", [P, TC, VOCAB], FP32, kind="ExternalOutput")

    with tile.TileContext(nc) as tc, ExitStack() as ctx:
        sb = ctx.enter_context(tc.tile_pool(name="sb", bufs=1))
        vpool = ctx.enter_context(tc.tile_pool(name="vpool", bufs=NVG))
        qxpool = ctx.enter_context(tc.tile_pool(name="qxpool", bufs=5))
        wpool = ctx.enter_context(tc.tile_pool(name="wpool", bufs=2))
        tabpool = ctx.enter_context(tc.tile_pool(name="tabpool", bufs=2))
        roppool = ctx.enter_context(tc.tile_pool(name="roppool", bufs=2))
        mixpool = ctx.enter_context(tc.tile_pool(name="mixpool", bufs=2))
        statpool = ctx.enter_context(tc.tile_pool(name="statpool", bufs=8))
        xpool = ctx.enter_context(tc.tile_pool(name="xpool", bufs=2))
        apsum = ctx.enter_context(tc.tile_pool(name="apsum", bufs=2, space="PSUM"))
        cpsum = ctx.enter_context(tc.tile_pool(name="cpsum", bufs=1, space="PSUM"))
        drm = ctx.enter_context(tc.tile_pool(name="drm", bufs=2, space="DRAM"))

        ident = sb.tile([P, P], BF, name="ident")
        make_identity(nc, ident)
        epst = sb.tile([P, 1], FP32, name="epst")
        nc.vector.memset(epst, EPS)
        # x_T first: layer 0's first matmuls wait only on it (+ enc tiles).
        x_T = xpool.tile([P, DT, T], BF, tag="xT", name="x_T0")
        nc.sync.dma_start(out=x_T, in_=x0T_d[:])
        x_bf = xpool.tile([P, TC, D], BF, tag="xbf", name="x_bf0")
        nc.sync.dma_start(out=x_bf, in_=x0bf_d[:])
        x_f = xpool.tile([P, TC, D], FP32, tag="xf", name="x_f0")
        nc.sync.dma_start(out=x_f, in_=x0_d[:])
        maskt = sb.tile([P, TC, T], BF, name="maskt")
        nc.sync.dma_start(out=maskt, in_=mask_d[:])
        lmt = sb.tile([P, DT, VOCAB], BF, name="lmt")
        nc.sync.dma_start(out=lmt, in_=lm_d[:])

        def layer_norm_stats(src_ap, name):
            """Returns (mv, rstd) where mv[:,0:1]=mean, rstd=1/sqrt(var+eps)."""
            stats = statpool.tile([P, 6], FP32, tag="bst", name=f"st_{name}")
            nc.vector.bn_stats(out=stats, in_=src_ap)
            mv = statpool.tile([P, 2], FP32, tag="bmv", name=f"mv_{name}")
            nc.vector.bn_aggr(out=mv, in_=stats)
            rstd = statpool.tile([P, 1], FP32, tag="brs", name=f"rs_{name}")
            nc.scalar.activation(out=rstd, in_=mv[:, 1:2], func=AF.Sqrt, bias=epst)
            nc.vector.reciprocal(rstd, rstd)
            return mv, rstd

        def layer_norm_stats_batch(srcs, name):
            """Stage-ordered LN stats for several chunks: all bn_stats+aggr
            first, then the ACT sqrts, then the DVE recips — avoids DVE
            head-of-line blocking on the ACT round-trip."""
            mvs, rstds = [], []
            for i, src in enumerate(srcs):
                stats = statpool.tile([P, 6], FP32, tag="bst", name=f"st_{name}{i}")
                nc.vector.bn_stats(out=stats, in_=src)
                mv = statpool.tile([P, 2], FP32, tag="bmv", name=f"mv_{name}{i}")
                nc.vector.bn_aggr(out=mv, in_=stats)
                mvs.append(mv)
            for i, mv in enumerate(mvs):
                rstd = statpool.tile([P, 1], FP32, tag="brs", name=f"rs_{name}{i}")
                nc.scalar.activation(
                    out=rstd, in_=mv[:, 1:2], func=AF.Sqrt, bias=epst
                )
                rstds.append(rstd)
            for rstd in rstds:
                nc.vector.reciprocal(rstd, rstd)
            return list(zip(mvs, rstds))

        def emit_layer(l, x_f, x_bf, x_T):
            # ---------------- step A: V^T = relu(enc^T @ x^T), [N, T] ------
            V = [None] * NVG

            def emit_A(vg, split=False):
                encg = wpool.tile([P, DT, VG, P], BF, tag="w", name=f"enc{l}_{vg}")
                nc.sync.dma_start(
                    out=encg,
                    in_=enc_d[:, :, ds(vg * VG, VG), :].rearrange(
                        "dt p nt n -> p dt nt n"
                    ),
                )
                vt = vpool.tile([P, VG, T], BF, tag="v", name=f"v{l}_{vg}")
                V[vg] = vt
                if split:
                    # Borrow the (currently idle) C-phase PSUM banks for two
                    # extra quads so all four quads' first-half matmuls can
                    # run before the second t-half of x_T is ready.
                    aq = cpsum.tile([P, TC, T], FP32, tag="mm", name=f"aq{l}")
                    q01 = [aq[:, 0:2, :], aq[:, 2:4, :]]
                    q23 = [
                        apsum.tile([P, 2, T], FP32, tag="quad", name=f"as{l}_{q}")
                        for q in range(2)
                    ]
                    quads = q01 + q23
                    for h in range(2):
                        for q in range(4):
                            for i in range(2):
                                for dt_ in range(DT):
                                    nc.tensor.matmul(
                                        quads[q][:, i, ds(h * HT, HT)],
                                        lhsT=encg[:, dt_, q * 2 + i, :],
                                        rhs=x_T[:, dt_, ds(h * HT, HT)],
                                        start=(dt_ == 0),
                                        stop=(dt_ == DT - 1),
                                    )
                    for q in range(4):
                        nc.scalar.activation(
                            out=vt[:, ds(q * 2, 2), :], in_=quads[q],
                            func=AF.Relu,
                        )
                    return
                for q in range(VG // 2):
                    ps = apsum.tile(
                        [P, 2, T], FP32, tag="quad", name=f"aps{l}_{vg}_{q}"
                    )
                    for i in range(2):
                        for dt_ in range(DT):
                            nc.tensor.matmul(
                                ps[:, i, :],
                                lhsT=encg[:, dt_, q * 2 + i, :],
                                rhs=x_T[:, dt_, :],
                                start=(dt_ == 0),
                                stop=(dt_ == DT - 1),
                            )
                    nc.scalar.activation(
                        out=vt[:, ds(q * 2, 2), :], in_=ps, func=AF.Relu
                    )

            # ---------------- rope: QR = V*cos + Vpartner*sin' -------------
            QR = [None] * NG

            def emit_rope(g):
                cosg = tabpool.tile([P, G4, T], BF, tag="cos", name=f"cos{l}_{g}")
                nc.sync.dma_start(out=cosg, in_=cos_d[:, ds(g * G4, G4), :])
                sing = tabpool.tile([P, G4, T], BF, tag="sin", name=f"sin{l}_{g}")
                nc.sync.dma_start(out=sing, in_=sin_d[:, ds(g * G4, G4), :])
                qr = qxpool.tile([P, G4, T], BF, tag="qx", name=f"qr{l}_{g}")
                QR[g] = qr
                pg = roppool.tile([P, G4, T], BF, tag="rp", name=f"rp{l}_{g}")
                p2 = roppool.tile([P, G4, T], BF, tag="rp2", name=f"rq{l}_{g}")
                vg_, off = divmod(g * G4, VG)
                pvg_, poff = divmod((g ^ (NG // 2)) * G4, VG)
                nc.vector.tensor_mul(pg, V[vg_][:, ds(off, G4), :], cosg)
                nc.vector.tensor_mul(p2, V[pvg_][:, ds(poff, G4), :], sing)
                nc.vector.tensor_add(qr, pg, p2)

            for pair in range(NVG // 2):
                emit_A(pair, split=(pair == 0))
                emit_A(pair + NVG // 2)
                emit_rope(pair * 2)
                emit_rope(pair * 2 + 1)
            for g in range(NG // 2, NG):
                emit_rope(g)

            # ---------------- step C: masked Gram in [s, t] ----------------
            gps = cpsum.tile([P, TC, T], FP32, tag="mm", name=f"gps{l}")
            for k in range(NT):
                g, i = divmod(k, G4)
                for j in range(TC):
                    nc.tensor.matmul(
                        gps[:, j, : T - j * P],
                        lhsT=QR[g][:, i, ds(j * P, P)],
                        rhs=QR[g][:, i, ds(j * P, T - j * P)],
                        start=(k == 0),
                        stop=(k == NT - 1),
                    )
            # PSUM -> SBUF cast: only the diagonal 128-blocks need the strict
            # mask (off-diagonal blocks are all-ones) — mask-mul them on
            # Vector while Scalar copies the off-diagonal spans in parallel.
            st = mixpool.tile([P, TC, T], BF, tag="st", name=f"st{l}")
            for j in range(TC):
                nc.vector.tensor_mul(
                    st[:, j, ds(j * P, P)],
                    gps[:, j, :P],
                    maskt[:, j, ds(j * P, P)],
                )
                if j < TC - 1:
                    nc.scalar.copy(
                        out=st[:, j, ds((j + 1) * P, T - (j + 1) * P)],
                        in_=gps[:, j, ds(P, T - (j + 1) * P)],
                    )

            # ---------------- step D: yKV = M^T @ x, then LN ---------------
            dps = cpsum.tile([P, TC, T], FP32, tag="mm", name=f"dps{l}")
            for jp in range(TC):
                for i in range(jp + 1):
                    nc.tensor.matmul(
                        dps[:, jp, :D],
                        lhsT=st[:, i, ds(jp * P, P)],
                        rhs=x_bf[:, i, :],
                        start=(i == 0),
                        stop=(i == jp),
                    )
            yln = mixpool.tile([P, TC, D], BF, tag="yln", name=f"yln{l}")
            dstats = layer_norm_stats_batch(
                [dps[:, jp, :D] for jp in range(TC)], f"d{l}"
            )
            for jp in range(TC):
                mv, rstd = dstats[jp]
                nc.vector.tensor_scalar(
                    out=yln[:, jp, :],
                    in0=dps[:, jp, :D],
                    scalar1=mv[:, 0:1],
                    scalar2=rstd,
                    op0=ALU.subtract,
                    op1=ALU.mult,
                )
            ylnT = mixpool.tile([P, DT, T], BF, tag="ylnT", name=f"ylnT{l}")
            for hv in range(2):
                tp = apsum.tile(
                    [P, DT, 2, P], BF, tag="quad", name=f"ytp{l}_{hv}"
                )
                for dt_ in range(DT):
                    for ji, jp in enumerate(range(hv * 2, hv * 2 + 2)):
                        nc.tensor.transpose(
                            tp[:, dt_, ji, :], yln[:, jp, ds(dt_ * P, P)], ident
                        )
                for dt_ in range(DT):
                    nc.scalar.copy(
                        out=ylnT[:, dt_, ds(hv * HT, HT)].rearrange(
                            "p (a b) -> p a b", a=2
                        ),
                        in_=tp[:, dt_],
                    )

            # ---------------- steps E+F fused: gated y_sparse + yMLP -------
            # F(g) staggered one group behind E(g); yMLP partial split over
            # n: P1 = groups 0..7, P2 = groups 8..15, accumulated into the
            # two column-halves of one C-phase PSUM tile.
            fpst = cpsum.tile([P, TC, T], FP32, tag="mm", name=f"fpst{l}")
            fhalf = [fpst[:, :, 0:D], fpst[:, :, ds(D, D)]]
            XY = [None] * NG
            EV = [None] * NVG

            def emit_E(g, split=False):
                vg, half = divmod(g, 2)
                if half == 0:
                    evg = wpool.tile(
                        [P, DT, VG, P], BF, tag="w", name=f"ev{l}_{vg}"
                    )
                    EV[vg] = evg
                    nc.sync.dma_start(
                        out=evg,
                        in_=encv_d[:, :, ds(vg * VG, VG), :].rearrange(
                            "dt p nt n -> p dt nt n"
                        ),
                    )
                evg = EV[vg]
                xy = qxpool.tile([P, G4, T], BF, tag="qx", name=f"xy{l}_{g}")
                XY[g] = xy
                for q in range(2):
                    ps = apsum.tile(
                        [P, 2, T], FP32, tag="quad", name=f"eps{l}_{g}_{q}"
                    )
                    if split:
                        for h in range(2):
                            for i in range(2):
                                nt_ = half * G4 + q * 2 + i
                                for dt_ in range(DT):
                                    nc.tensor.matmul(
                                        ps[:, i, ds(h * HT, HT)],
                                        lhsT=evg[:, dt_, nt_, :],
                                        rhs=ylnT[:, dt_, ds(h * HT, HT)],
                                        start=(dt_ == 0),
                                        stop=(dt_ == DT - 1),
                                    )
                    else:
                        for i in range(2):
                            nt_ = half * G4 + q * 2 + i
                            for dt_ in range(DT):
                                nc.tensor.matmul(
                                    ps[:, i, :],
                                    lhsT=evg[:, dt_, nt_, :],
                                    rhs=ylnT[:, dt_, :],
                                    start=(dt_ == 0),
                                    stop=(dt_ == DT - 1),
                                )
                    ys = roppool.tile(
                        [P, 2, T], BF, tag="rp2", name=f"ys{l}_{g}_{q}"
                    )
                    nc.scalar.activation(out=ys, in_=ps, func=AF.Relu)
                    nc.vector.tensor_mul(
                        xy[:, ds(q * 2, 2), :],
                        ys,
                        V[vg][:, ds(half * G4 + q * 2, 2), :],
                    )

            def emit_F(g, m_outer=False):
                decg = wpool.tile([P, G4, D], BF, tag="dec", name=f"dec{l}_{g}")
                nc.sync.dma_start(out=decg, in_=dec_d[:, ds(g * G4, G4), :])
                tgt = fhalf[g // 8]
                loop = (
                    [(i, m) for m in range(TC) for i in range(G4)]
                    if m_outer
                    else [(i, m) for i in range(G4) for m in range(TC)]
                )
                for i, m in loop:
                    kk = (g % 8) * G4 + i
                    nc.tensor.matmul(
                        tgt[:, m, :],
                        lhsT=XY[g][:, i, ds(m * P, P)],
                        rhs=decg[:, i, :],
                        start=(kk == 0),
                        stop=(kk == NT // 2 - 1),
                    )

            ccin1 = drm.tile([P, TC, D], BF, tag="ccin1", name=f"ccin1_{l}")
            ccout1 = drm.tile([P, TC, D], BF, tag="ccout1", name=f"ccout1_{l}")
            ymr1 = mixpool.tile([P, TC, D], BF, tag="ymr1", name=f"ymr1_{l}")
            for g in range(NG):
                emit_E(g, split=(g < 2))
                if g >= 2:
                    emit_F(g - 2)
                if g == 9:
                    # P1 complete: launch AllReduce #1 under the remaining
                    # E/F matmul stream.
                    ym1 = mixpool.tile([P, TC, D], BF, tag="ym1", name=f"ym1_{l}")
                    nc.scalar.copy(out=ym1, in_=fhalf[0])
                    nc.sync.dma_start(out=ccin1[:], in_=ym1)
                    nc.gpsimd.collective_compute(
                        "AllReduce",
                        ALU.add,
                        replica_groups=GROUPS,
                        ins=[ccin1[:]],
                        outs=[ccout1[:]],
                    )
                    nc.sync.dma_start(out=ymr1, in_=ccout1[:])
            emit_F(NG - 2)
            emit_F(NG - 1, m_outer=True)

            # P2 complete: two chunk-pair AllReduces (copies on Scalar — it
            # wakes instantly after the accumulation stop, unlike Vector).
            ymr2 = [None, None]
            for hv in range(2):
                ym2 = mixpool.tile(
                    [P, 2, D], BF, tag=f"ym2{hv}", name=f"ym2_{l}_{hv}"
                )
                nc.scalar.copy(out=ym2, in_=fhalf[1][:, ds(hv * 2, 2), :])
                cc_in = drm.tile(
                    [P, 2, D], BF, tag=f"ccin2{hv}", name=f"ccin2_{l}_{hv}"
                )
                cc_out = drm.tile(
                    [P, 2, D], BF, tag=f"ccout2{hv}", name=f"ccout2_{l}_{hv}"
                )
                nc.sync.dma_start(out=cc_in[:], in_=ym2)
                nc.gpsimd.collective_compute(
                    "AllReduce",
                    ALU.add,
                    replica_groups=GROUPS,
                    ins=[cc_in[:]],
                    outs=[cc_out[:]],
                )
                ymr2[hv] = mixpool.tile(
                    [P, 2, D], BF, tag=f"ymr2{hv}", name=f"ymr2_{l}_{hv}"
                )
                nc.sync.dma_start(out=ymr2[hv], in_=cc_out[:])

            # ---------------- x = LN(x + LN(yMLP)), per chunk-pair ---------
            x_f_new = xpool.tile([P, TC, D], FP32, tag="xf", name=f"x_f{l + 1}")
            x_bf_new = xpool.tile([P, TC, D], BF, tag="xbf", name=f"x_bf{l + 1}")
            x_T_new = xpool.tile([P, DT, T], BF, tag="xT", name=f"x_T{l + 1}")
            xmid = mixpool.tile([P, TC, D], FP32, tag="xmid", name=f"xm{l}")
            last = l == N_LAYER - 1
            if last:
                lps = cpsum.tile([P, TC, T], FP32, tag="mm", name="lps")
                lout = mixpool.tile([P, TC, VOCAB], FP32, tag="lout", name="lout")
            ysum = mixpool.tile([P, TC, D], BF, tag="ysum", name=f"ys{l}")
            for hv in range(2):
                jps = list(range(hv * 2, hv * 2 + 2))
                nc.vector.tensor_add(
                    ysum[:, ds(hv * 2, 2), :],
                    ymr1[:, ds(hv * 2, 2), :],
                    ymr2[hv],
                )
                ystats = layer_norm_stats_batch(
                    [ysum[:, jp, :] for jp in jps], f"y{l}_{hv}"
                )
                for ji, jp in enumerate(jps):
                    nc.vector.scalar_tensor_tensor(
                        out=xmid[:, jp, :],
                        in0=ysum[:, jp, :],
                        scalar=ystats[ji][1],
                        in1=x_f[:, jp, :],
                        op0=ALU.mult,
                        op1=ALU.add,
                    )
                xstats = layer_norm_stats_batch(
                    [xmid[:, jp, :] for jp in jps], f"x{l}_{hv}"
                )
                for ji, jp in enumerate(jps):
                    mv2, r2 = xstats[ji]
                    nc.vector.tensor_scalar(
                        out=x_bf_new[:, jp, :],
                        in0=xmid[:, jp, :],
                        scalar1=mv2[:, 0:1],
                        scalar2=r2,
                        op0=ALU.subtract,
                        op1=ALU.mult,
                    )
                # One PSUM tile per chunk-pair for all four transposes —
                # halves the quad-ring pressure so the next layer's split-A
                # quads are not WAR-blocked behind the hv=1 transposes.
                tp = apsum.tile(
                    [P, DT, 2, P], BF, tag="quad", name=f"xtp{l}_{hv}"
                )
                for dt_ in range(DT):
                    for ji, jp in enumerate(jps):
                        nc.tensor.transpose(
                            tp[:, dt_, ji, :],
                            x_bf_new[:, jp, ds(dt_ * P, P)],
                            ident,
                        )
                for dt_ in range(DT):
                    nc.scalar.copy(
                        out=x_T_new[:, dt_, ds(hv * HT, HT)].rearrange(
                            "p (a b) -> p a b", a=2
                        ),
                        in_=tp[:, dt_],
                    )
                if last:
                    # lm head folded into the final boundary, per chunk-pair.
                    for jp in jps:
                        for dt_ in range(DT):
                            nc.tensor.matmul(
                                lps[:, jp, :VOCAB],
                                lhsT=x_T_new[:, dt_, ds(jp * P, P)],
                                rhs=lmt[:, dt_, :],
                                start=(dt_ == 0),
                                stop=(dt_ == DT - 1),
                            )
                    nc.scalar.copy(
                        out=lout[:, ds(hv * 2, 2), :],
                        in_=lps[:, ds(hv * 2, 2), :VOCAB],
                    )
                    nc.sync.dma_start(
                        out=out_d[:, ds(hv * 2, 2), :],
                        in_=lout[:, ds(hv * 2, 2), :],
                    )
                else:
                    for ji, jp in enumerate(jps):
                        mv2, r2 = xstats[ji]
                        nc.vector.tensor_scalar(
                            out=x_f_new[:, jp, :],
                            in0=xmid[:, jp, :],
                            scalar1=mv2[:, 0:1],
                            scalar2=r2,
                            op0=ALU.subtract,
                            op1=ALU.mult,
                        )
            return x_f_new, x_bf_new, x_T_new

        for l in range(N_LAYER):
            x_f, x_bf, x_T = emit_layer(l, x_f, x_bf, x_T)

    if not nc.is_finalized():
        nc.finalize()
    return nc


def _ln_np(x):
    m = x.mean(-1, keepdims=True)
    v = ((x - m) ** 2).mean(-1, keepdims=True)
    return (x - m) / np.sqrt(v + EPS)


def _make_tables():
    t = np.arange(N, dtype=np.float32)
    q = np.floor(t / 2.0) * 2.0
    freqs = (1.0 / (THETA ** (q / N)) / (2.0 * np.float32(math.pi))).astype(
        np.float32
    )
    phases = np.arange(T, dtype=np.float32)[:, None] * freqs[None, :]
    ph = np.float32(np.float32(phases % 1.0) * np.float32(2.0 * math.pi))
    return np.cos(ph).astype(np.float32), np.sin(ph).astype(np.float32)


def _prep_inputs(idx, embed_w, encoder, encoder_v, decoder, lm_head):
    perm = np.concatenate([np.arange(HALF) * 2, np.arange(HALF) * 2 + 1])

    cos, sin = _make_tables()
    cosp = cos[:, perm]
    sinp = sin[:, perm].copy()
    sinp[:, :HALF] *= -1.0
    # [P, NT, T]: (p, nt, t) -> table[t, nt*P + p]
    cos_h = np.ascontiguousarray(
        cosp.T.reshape(NT, P, T).transpose(1, 0, 2)
    ).astype(BF16)
    sin_h = np.ascontiguousarray(
        sinp.T.reshape(NT, P, T).transpose(1, 0, 2)
    ).astype(BF16)

    mask_h = np.zeros((P, TC, T), np.float32)
    t_idx = np.arange(T)
    for j in range(TC):
        for p in range(P):
            mask_h[p, j] = (t_idx > (j * P + p)).astype(np.float32)
    mask_h = mask_h.astype(BF16)

    lm_h = np.ascontiguousarray(
        lm_head.reshape(DT, P, VOCAB).transpose(1, 0, 2)
    ).astype(BF16)

    x0 = _ln_np(embed_w[idx].astype(np.float32))  # (B, T, D)

    dec3 = decoder.reshape(NH, N, D)

    per_core = []
    for core in range(N_CORES):
        b, h = divmod(core, NH)
        enc_p = encoder[h][:, perm]  # (D, N)
        encv_p = encoder_v[h][:, perm]
        dec_p = dec3[h][perm, :]  # (N, D)

        enc_h = enc_p.reshape(DT, P, NT, P).astype(BF16)
        encv_h = encv_p.reshape(DT, P, NT, P).astype(BF16)
        dec_h = np.ascontiguousarray(
            dec_p.reshape(NT, P, D).transpose(1, 0, 2)
        ).astype(BF16)

        xb = x0[b]  # (T, D) f32
        x0_c = np.ascontiguousarray(
            xb.reshape(TC, P, D).transpose(1, 0, 2)
        ).astype(np.float32)
        x0bf_c = x0_c.astype(BF16)
        x0T_c = np.ascontiguousarray(
            xb.T.reshape(DT, P, T).transpose(1, 0, 2)
        ).astype(BF16)

        per_core.append(
            {
                "x0": x0_c,
                "x0bf": x0bf_c,
                "x0T": x0T_c,
                "enc": enc_h,
                "encv": encv_h,
                "dec": dec_h,
                "cosb": cos_h,
                "sinb": sin_h,
                "maskb": mask_h,
                "lm": lm_h,
            }
        )
    return per_core


def _get_nc():
    if "nc" not in _CACHE:
        _CACHE["nc"] = _build_bass()
    return _CACHE["nc"]


def kernel(idx, embed_w, encoder, encoder_v, decoder, lm_head, **extra):
    idx = np.asarray(idx)
    embed_w = np.asarray(embed_w, dtype=np.float32)
    encoder = np.asarray(encoder, dtype=np.float32)
    encoder_v = np.asarray(encoder_v, dtype=np.float32)
    decoder = np.asarray(decoder, dtype=np.float32)
    lm_head = np.asarray(lm_head, dtype=np.float32)

    nc = _get_nc()
    in_maps = _prep_inputs(idx, embed_w, encoder, encoder_v, decoder, lm_head)
    res = run_bass_kernel_spmd(nc, in_maps, core_ids=list(range(N_CORES)))
    _CACHE["last_results"] = res

    out = np.zeros((B, T, VOCAB), np.float32)
    for b in range(B):
        lg = res.results[b * NH]["logits"]  # [P, TC, VOCAB]
        out[b] = lg.transpose(1, 0, 2).reshape(T, VOCAB)
    return out


if __name__ == "__main__":
    rng = np.random.default_rng(0)
    ins = {
        "idx": rng.integers(0, VOCAB, (B, T)).astype(np.int32),
        "embed_w": (0.02 * rng.standard_normal((VOCAB, D))).astype(np.float32),
        "encoder": (0.02 * rng.standard_normal((NH, D, N))).astype(np.float32),
        "encoder_v": (0.02 * rng.standard_normal((NH, D, N))).astype(np.float32),
        "decoder": (0.02 * rng.standard_normal((NH * N, D))).astype(np.float32),
        "lm_head": (0.02 * rng.standard_normal((D, VOCAB))).astype(np.float32),
    }
    out = kernel(**ins)
    print("out", out.shape, out.dtype, float(np.abs(out).max()))


# revision 46
# speedup vs baseline: 1.0770x; 1.0406x over previous
"""Trainium2 Bass kernel for nn_BDH_39127152067244 (dense_transformer).

Sharding: 8 cores = (b, h) pairs — b = core // 4, h = core % 4. Each core
computes its head's share of every layer; the only cross-core communication
is AllReduce of the per-head yMLP partial [T, D] once per layer
(replica groups {0..3} and {4..7}).

v2 layer pipeline (vs v1):
  - yMLP partial is split over the sparse dim N into two halves (P1 =
    n-tiles 0..31, P2 = 32..63). AllReduce(P1) launches halfway through
    the fused E/F phase and hides under ~27us of remaining matmuls;
    AllReduce(P2) is split into two chunk-pair collectives so the
    LN + transpose chain for t-chunks (0,1) overlaps the second one.
  - F(g) is staggered one group behind E(g) so the relu+gate chain for
    XY[g] never stalls the PE.
  - The first A group of the next layer is t-half split (and borrows the
    C-phase PSUM banks for 4 concurrent quads) so the PE restarts as soon
    as half of the new x is normalized.
  - rope's partner multiply runs on GpSimd, one relu quad per A group on
    Vector, the rest on Scalar — balancing the three elementwise engines
    against the (GPIO-throttled) PE streaming rate.
  - LayerNorm rstd uses a single Rsqrt activation.

Layout tricks (unchanged from v1):
  - The N axis (8192) is deinterleaved on the host (even n first, odd n
    second), applied consistently to encoder / encoder_v / decoder rows and
    the rope tables. Rope's interleaved pair-swap then becomes a clean
    half-offset of whole 128-partition tiles with a sign folded into the
    sin table.
  - x_sparse is computed directly in transposed [N, T] layout; the masked
    Gram matrix is computed directly as the yKV matmul's lhsT in [s, t]
    layout (symmetry), skipping fully-masked tiles.
  - All matmuls run in bf16 with f32 PSUM accumulation; LayerNorms and the
    residual stream stay f32.
"""

import math
import sys
from contextlib import ExitStack

import numpy as np
import ml_dtypes

sys.path.insert(0, "/opt/trn_rl_repo")

import concourse.bass as bass  # noqa: E402
import concourse.bacc as bacc  # noqa: E402
import concourse.mybir as mybir  # noqa: E402
import concourse.tile as tile  # noqa: E402
from concourse.bass import ds  # noqa: E402
from concourse.bass_utils import run_bass_kernel_spmd  # noqa: E402
from concourse.masks import make_identity  # noqa: E402

BF16 = ml_dtypes.bfloat16
BF = mybir.dt.bfloat16
FP32 = mybir.dt.float32
AF = mybir.ActivationFunctionType
ALU = mybir.AluOpType

# Problem constants (hardcoded per the harness contract).
N_LAYER = 6
D = 256
NH = 4
N = 8192
HALF = N // 2
VOCAB = 256
B, T = 2, 512
THETA = 2.0**16
EPS = 1e-5

P = 128          # partitions
HT = T // 2      # 256: t-half for the split A groups
NT = N // P      # 64 n-tiles
G4 = 4           # n-tiles per rope/qx group
NG = NT // G4    # 16 groups
VG = 8           # n-tiles per V tile
NVG = NT // VG   # 8 V tiles
TC = T // P      # 4 t-chunks
DT = D // P      # 2 d-tiles
N_CORES = 8
GROUPS = [[0, 1, 2, 3], [4, 5, 6, 7]]

_CACHE: dict = {}


def _build_bass():
    nc = bacc.Bacc("TRN2", num_devices=N_CORES)

    x0_d = nc.dram_tensor("x0", [P, TC, D], FP32, kind="ExternalInput")
    x0bf_d = nc.dram_tensor("x0bf", [P, TC, D], BF, kind="ExternalInput")
    x0T_d = nc.dram_tensor("x0T", [P, DT, T], BF, kind="ExternalInput")
    enc_d = nc.dram_tensor("enc", [DT, P, NT, P], BF, kind="ExternalInput")
    encv_d = nc.dram_tensor("encv", [DT, P, NT, P], BF, kind="ExternalInput")
    dec_d = nc.dram_tensor("dec", [P, NT, D], BF, kind="ExternalInput")
    cos_d = nc.dram_tensor("cosb", [P, NT, T], BF, kind="ExternalInput")
    sin_d = nc.dram_tensor("sinb", [P, NT, T], BF, kind="ExternalInput")
    mask_d = nc.dram_tensor("maskb", [P, TC, T], BF, kind="ExternalInput")
    lm_d = nc.dram_tensor("lm", [P, DT, VOCAB], BF, kind="ExternalInput")
    out_d = nc.dram_tensor("logits", [P, TC, VOCAB], FP32, kind="ExternalOutput")

    with tile.TileContext(nc) as tc, ExitStack() as ctx:
        sb = ctx.enter_context(tc.tile_pool(name="sb", bufs=1))
        vpool = ctx.enter_context(tc.tile_pool(name="vpool", bufs=NVG))
        qxpool = ctx.enter_context(tc.tile_pool(name="qxpool", bufs=5))
        wpool = ctx.enter_context(tc.tile_pool(name="wpool", bufs=2))
        tabpool = ctx.enter_context(tc.tile_pool(name="tabpool", bufs=2))
        roppool = ctx.enter_context(tc.tile_pool(name="roppool", bufs=2))
        mixpool = ctx.enter_context(tc.tile_pool(name="mixpool", bufs=2))
        statpool = ctx.enter_context(tc.tile_pool(name="statpool", bufs=8))
        xpool = ctx.enter_context(tc.tile_pool(name="xpool", bufs=2))
        apsum = ctx.enter_context(tc.tile_pool(name="apsum", bufs=2, space="PSUM"))
        cpsum = ctx.enter_context(tc.tile_pool(name="cpsum", bufs=1, space="PSUM"))
        drm = ctx.enter_context(tc.tile_pool(name="drm", bufs=2, space="DRAM"))

        ident = sb.tile([P, P], BF, name="ident")
        make_identity(nc, ident)
        epst = sb.tile([P, 1], FP32, name="epst")
        nc.vector.memset(epst, EPS)
        # x_T first: layer 0's first matmuls wait only on it (+ enc tiles).
        x_T = xpool.tile([P, DT, T], BF, tag="xT", name="x_T0")
        nc.sync.dma_start(out=x_T, in_=x0T_d[:])
        x_bf = xpool.tile([P, TC, D], BF, tag="xbf", name="x_bf0")
        nc.sync.dma_start(out=x_bf, in_=x0bf_d[:])
        x_f = xpool.tile([P, TC, D], FP32, tag="xf", name="x_f0")
        nc.sync.dma_start(out=x_f, in_=x0_d[:])
        maskt = sb.tile([P, TC, T], BF, name="maskt")
        nc.sync.dma_start(out=maskt, in_=mask_d[:])
        lmt = sb.tile([P, DT, VOCAB], BF, name="lmt")
        nc.sync.dma_start(out=lmt, in_=lm_d[:])

        def layer_norm_stats(src_ap, name):
            """Returns (mv, rstd) where mv[:,0:1]=mean, rstd=1/sqrt(var+eps)."""
            stats = statpool.tile([P, 6], FP32, tag="bst", name=f"st_{name}")
            nc.vector.bn_stats(out=stats, in_=src_ap)
            mv = statpool.tile([P, 2], FP32, tag="bmv", name=f"mv_{name}")
            nc.vector.bn_aggr(out=mv, in_=stats)
            rstd = statpool.tile([P, 1], FP32, tag="brs", name=f"rs_{name}")
            nc.scalar.activation(out=rstd, in_=mv[:, 1:2], func=AF.Sqrt, bias=epst)
            nc.vector.reciprocal(rstd, rstd)
            return mv, rstd

        def layer_norm_stats_batch(srcs, name):
            """Stage-ordered LN stats for several chunks: all bn_stats+aggr
            first, then the ACT sqrts, then the DVE recips — avoids DVE
            head-of-line blocking on the ACT round-trip."""
            mvs, rstds = [], []
            for i, src in enumerate(srcs):
                stats = statpool.tile([P, 6], FP32, tag="bst", name=f"st_{name}{i}")
                nc.vector.bn_stats(out=stats, in_=src)
                mv = statpool.tile([P, 2], FP32, tag="bmv", name=f"mv_{name}{i}")
                nc.vector.bn_aggr(out=mv, in_=stats)
                mvs.append(mv)
            for i, mv in enumerate(mvs):
                rstd = statpool.tile([P, 1], FP32, tag="brs", name=f"rs_{name}{i}")
                nc.scalar.activation(
                    out=rstd, in_=mv[:, 1:2], func=AF.Sqrt, bias=epst
                )
                rstds.append(rstd)
            for rstd in rstds:
                nc.vector.reciprocal(rstd, rstd)
            return list(zip(mvs, rstds))

        def emit_layer(l, x_f, x_bf, x_T):
            # ---------------- step A: V^T = relu(enc^T @ x^T), [N, T] ------
            V = [None] * NVG

            def emit_A(vg, split=False):
                encg = wpool.tile([P, DT, VG, P], BF, tag="w", name=f"enc{l}_{vg}")
                nc.sync.dma_start(
                    out=encg,
                    in_=enc_d[:, :, ds(vg * VG, VG), :].rearrange(
                        "dt p nt n -> p dt nt n"
                    ),
                )
                vt = vpool.tile([P, VG, T], BF, tag="v", name=f"v{l}_{vg}")
                V[vg] = vt
                if split:
                    # Borrow the (currently idle) C-phase PSUM banks for two
                    # extra quads so all four quads' first-half matmuls can
                    # run before the second t-half of x_T is ready.
                    aq = cpsum.tile([P, TC, T], FP32, tag="mm", name=f"aq{l}")
                    q01 = [aq[:, 0:2, :], aq[:, 2:4, :]]
                    q23 = [
                        apsum.tile([P, 2, T], FP32, tag="quad", name=f"as{l}_{q}")
                        for q in range(2)
                    ]
                    quads = q01 + q23
                    for h in range(2):
                        for q in range(4):
                            for i in range(2):
                                for dt_ in range(DT):
                                    nc.tensor.matmul(
                                        quads[q][:, i, ds(h * HT, HT)],
                                        lhsT=encg[:, dt_, q * 2 + i, :],
                                        rhs=x_T[:, dt_, ds(h * HT, HT)],
                                        start=(dt_ == 0),
                                        stop=(dt_ == DT - 1),
                                    )
                    for q in range(4):
                        nc.scalar.activation(
                            out=vt[:, ds(q * 2, 2), :], in_=quads[q],
                            func=AF.Relu,
                        )
                    return
                for q in range(VG // 2):
                    ps = apsum.tile(
                        [P, 2, T], FP32, tag="quad", name=f"aps{l}_{vg}_{q}"
                    )
                    for i in range(2):
                        for dt_ in range(DT):
                            nc.tensor.matmul(
                                ps[:, i, :],
                                lhsT=encg[:, dt_, q * 2 + i, :],
                                rhs=x_T[:, dt_, :],
                                start=(dt_ == 0),
                                stop=(dt_ == DT - 1),
                            )
                    nc.scalar.activation(
                        out=vt[:, ds(q * 2, 2), :], in_=ps, func=AF.Relu
                    )

            # ---------------- rope: QR = V*cos + Vpartner*sin' -------------
            QR = [None] * NG

            def emit_rope(g):
                cosg = tabpool.tile([P, G4, T], BF, tag="cos", name=f"cos{l}_{g}")
                nc.sync.dma_start(out=cosg, in_=cos_d[:, ds(g * G4, G4), :])
                sing = tabpool.tile([P, G4, T], BF, tag="sin", name=f"sin{l}_{g}")
                nc.sync.dma_start(out=sing, in_=sin_d[:, ds(g * G4, G4), :])
                qr = qxpool.tile([P, G4, T], BF, tag="qx", name=f"qr{l}_{g}")
                QR[g] = qr
                pg = roppool.tile([P, G4, T], BF, tag="rp", name=f"rp{l}_{g}")
                p2 = roppool.tile([P, G4, T], BF, tag="rp2", name=f"rq{l}_{g}")
                vg_, off = divmod(g * G4, VG)
                pvg_, poff = divmod((g ^ (NG // 2)) * G4, VG)
                nc.vector.tensor_mul(pg, V[vg_][:, ds(off, G4), :], cosg)
                nc.vector.tensor_mul(p2, V[pvg_][:, ds(poff, G4), :], sing)
                nc.vector.tensor_add(qr, pg, p2)

            for pair in range(NVG // 2):
                emit_A(pair, split=(pair == 0))
                emit_A(pair + NVG // 2)
                emit_rope(pair * 2)
                emit_rope(pair * 2 + 1)
            for g in range(NG // 2, NG):
                emit_rope(g)

            # ---------------- step C: masked Gram in [s, t] ----------------
            gps = cpsum.tile([P, TC, T], FP32, tag="mm", name=f"gps{l}")
            for k in range(NT):
                g, i = divmod(k, G4)
                for j in range(TC):
                    nc.tensor.matmul(
                        gps[:, j, : T - j * P],
                        lhsT=QR[g][:, i, ds(j * P, P)],
                        rhs=QR[g][:, i, ds(j * P, T - j * P)],
                        start=(k == 0),
                        stop=(k == NT - 1),
                    )
            # PSUM -> SBUF cast: only the diagonal 128-blocks need the strict
            # mask (off-diagonal blocks are all-ones) — mask-mul them on
            # Vector while Scalar copies the off-diagonal spans in parallel.
            st = mixpool.tile([P, TC, T], BF, tag="st", name=f"st{l}")
            for j in range(TC):
                nc.vector.tensor_mul(
                    st[:, j, ds(j * P, P)],
                    gps[:, j, :P],
                    maskt[:, j, ds(j * P, P)],
                )
                if j < TC - 1:
                    nc.scalar.copy(
                        out=st[:, j, ds((j + 1) * P, T - (j + 1) * P)],
                        in_=gps[:, j, ds(P, T - (j + 1) * P)],
                    )

            # ---------------- step D: yKV = M^T @ x, then LN ---------------
            dps = cpsum.tile([P, TC, T], FP32, tag="mm", name=f"dps{l}")
            for jp in range(TC):
                for i in range(jp + 1):
                    nc.tensor.matmul(
                        dps[:, jp, :D],
                        lhsT=st[:, i, ds(jp * P, P)],
                        rhs=x_bf[:, i, :],
                        start=(i == 0),
                        stop=(i == jp),
                    )
            yln = mixpool.tile([P, TC, D], BF, tag="yln", name=f"yln{l}")
            dstats = layer_norm_stats_batch(
                [dps[:, jp, :D] for jp in range(TC)], f"d{l}"
            )
            for jp in range(TC):
                mv, rstd = dstats[jp]
                nc.vector.tensor_scalar(
                    out=yln[:, jp, :],
                    in0=dps[:, jp, :D],
                    scalar1=mv[:, 0:1],
                    scalar2=rstd,
                    op0=ALU.subtract,
                    op1=ALU.mult,
                )
            ylnT = mixpool.tile([P, DT, T], BF, tag="ylnT", name=f"ylnT{l}")
            for hv in range(2):
                tp = apsum.tile(
                    [P, DT, 2, P], BF, tag="quad", name=f"ytp{l}_{hv}"
                )
                for dt_ in range(DT):
                    for ji, jp in enumerate(range(hv * 2, hv * 2 + 2)):
                        nc.tensor.transpose(
                            tp[:, dt_, ji, :], yln[:, jp, ds(dt_ * P, P)], ident
                        )
                for dt_ in range(DT):
                    nc.scalar.copy(
                        out=ylnT[:, dt_, ds(hv * HT, HT)].rearrange(
                            "p (a b) -> p a b", a=2
                        ),
                        in_=tp[:, dt_],
                    )

            # ---------------- steps E+F fused: gated y_sparse + yMLP -------
            # F(g) staggered one group behind E(g); yMLP partial split over
            # n: P1 = groups 0..7, P2 = groups 8..15, accumulated into the
            # two column-halves of one C-phase PSUM tile.
            fpst = cpsum.tile([P, TC, T], FP32, tag="mm", name=f"fpst{l}")
            fhalf = [fpst[:, :, 0:D], fpst[:, :, ds(D, D)]]
            XY = [None] * NG
            EV = [None] * NVG

            def emit_E(g, split=False):
                vg, half = divmod(g, 2)
                if half == 0:
                    evg = wpool.tile(
                        [P, DT, VG, P], BF, tag="w", name=f"ev{l}_{vg}"
                    )
                    EV[vg] = evg
                    nc.sync.dma_start(
                        out=evg,
                        in_=encv_d[:, :, ds(vg * VG, VG), :].rearrange(
                            "dt p nt n -> p dt nt n"
                        ),
                    )
                evg = EV[vg]
                xy = qxpool.tile([P, G4, T], BF, tag="qx", name=f"xy{l}_{g}")
                XY[g] = xy
                for q in range(2):
                    ps = apsum.tile(
                        [P, 2, T], FP32, tag="quad", name=f"eps{l}_{g}_{q}"
                    )
                    if split:
                        for h in range(2):
                            for i in range(2):
                                nt_ = half * G4 + q * 2 + i
                                for dt_ in range(DT):
                                    nc.tensor.matmul(
                                        ps[:, i, ds(h * HT, HT)],
                                        lhsT=evg[:, dt_, nt_, :],
                                        rhs=ylnT[:, dt_, ds(h * HT, HT)],
                                        start=(dt_ == 0),
                                        stop=(dt_ == DT - 1),
                                    )
                    else:
                        for i in range(2):
                            nt_ = half * G4 + q * 2 + i
                            for dt_ in range(DT):
                                nc.tensor.matmul(
                                    ps[:, i, :],
                                    lhsT=evg[:, dt_, nt_, :],
                                    rhs=ylnT[:, dt_, :],
                                    start=(dt_ == 0),
                                    stop=(dt_ == DT - 1),
                                )
                    ys = roppool.tile(
                        [P, 2, T], BF, tag="rp2", name=f"ys{l}_{g}_{q}"
                    )
                    nc.scalar.activation(out=ys, in_=ps, func=AF.Relu)
                    nc.vector.tensor_mul(
                        xy[:, ds(q * 2, 2), :],
                        ys,
                        V[vg][:, ds(half * G4 + q * 2, 2), :],
                    )

            def emit_F(g, m_outer=False):
                decg = wpool.tile([P, G4, D], BF, tag="dec", name=f"dec{l}_{g}")
                nc.sync.dma_start(out=decg, in_=dec_d[:, ds(g * G4, G4), :])
                tgt = fhalf[g // 8]
                loop = (
                    [(i, m) for m in range(TC) for i in range(G4)]
                    if m_outer
                    else [(i, m) for i in range(G4) for m in range(TC)]
                )
                for i, m in loop:
                    kk = (g % 8) * G4 + i
                    nc.tensor.matmul(
                        tgt[:, m, :],
                        lhsT=XY[g][:, i, ds(m * P, P)],
                        rhs=decg[:, i, :],
                        start=(kk == 0),
                        stop=(kk == NT // 2 - 1),
                    )

            ccin1 = drm.tile([P, TC, D], BF, tag="ccin1", name=f"ccin1_{l}")
            ccout1 = drm.tile([P, TC, D], BF, tag="ccout1", name=f"ccout1_{l}")
            ymr1 = mixpool.tile([P, TC, D], BF, tag="ymr1", name=f"ymr1_{l}")
            for g in range(NG):
                emit_E(g, split=(g < 2))
                if g >= 2:
                    emit_F(g - 2)
                if g == 9:
                    # P1 complete: launch AllReduce #1 under the remaining
                    # E/F matmul stream.
                    ym1 = mixpool.tile([P, TC, D], BF, tag="ym1", name=f"ym1_{l}")
                    nc.scalar.copy(out=ym1, in_=fhalf[0])
                    nc.sync.dma_start(out=ccin1[:], in_=ym1)
                    nc.gpsimd.collective_compute(
                        "AllReduce",
                        ALU.add,
                        replica_groups=GROUPS,
                        ins=[ccin1[:]],
                        outs=[ccout1[:]],
                    )
                    nc.sync.dma_start(out=ymr1, in_=ccout1[:])
            emit_F(NG - 2)
            emit_F(NG - 1, m_outer=True)

            # P2 complete: two chunk-pair AllReduces (copies on Scalar — it
            # wakes instantly after the accumulation stop, unlike Vector).
            # On the last layer there is no next-layer work to overlap the
            # second collective, so one merged AllReduce (which finishes
            # earlier than a serialized pair) shortens the tail instead.
            last_l = l == N_LAYER - 1
            if last_l:
                ym2 = mixpool.tile([P, TC, D], BF, tag="ym1", name=f"ym2_{l}")
                nc.scalar.copy(out=ym2, in_=fhalf[1])
                cc_in = drm.tile([P, TC, D], BF, tag="ccin1", name=f"ccin2_{l}")
                cc_out = drm.tile([P, TC, D], BF, tag="ccout1", name=f"ccout2_{l}")
                nc.sync.dma_start(out=cc_in[:], in_=ym2)
                nc.gpsimd.collective_compute(
                    "AllReduce",
                    ALU.add,
                    replica_groups=GROUPS,
                    ins=[cc_in[:]],
                    outs=[cc_out[:]],
                )
                ymr2m = mixpool.tile([P, TC, D], BF, tag="ymr1", name=f"ymr2_{l}")
                nc.sync.dma_start(out=ymr2m, in_=cc_out[:])
                ymr2 = [ymr2m[:, 0:2, :], ymr2m[:, 2:4, :]]
            else:
                ymr2 = [None, None]
            for hv in range(0 if last_l else 2):
                ym2 = mixpool.tile(
                    [P, 2, D], BF, tag=f"ym2{hv}", name=f"ym2_{l}_{hv}"
                )
                nc.scalar.copy(out=ym2, in_=fhalf[1][:, ds(hv * 2, 2), :])
                cc_in = drm.tile(
                    [P, 2, D], BF, tag=f"ccin2{hv}", name=f"ccin2_{l}_{hv}"
                )
                cc_out = drm.tile(
                    [P, 2, D], BF, tag=f"ccout2{hv}", name=f"ccout2_{l}_{hv}"
                )
                nc.sync.dma_start(out=cc_in[:], in_=ym2)
                nc.gpsimd.collective_compute(
                    "AllReduce",
                    ALU.add,
                    replica_groups=GROUPS,
                    ins=[cc_in[:]],
                    outs=[cc_out[:]],
                )
                ymr2[hv] = mixpool.tile(
                    [P, 2, D], BF, tag=f"ymr2{hv}", name=f"ymr2_{l}_{hv}"
                )
                nc.sync.dma_start(out=ymr2[hv], in_=cc_out[:])

            # ---------------- x = LN(x + LN(yMLP)), per chunk-pair ---------
            x_f_new = xpool.tile([P, TC, D], FP32, tag="xf", name=f"x_f{l + 1}")
            x_bf_new = xpool.tile([P, TC, D], BF, tag="xbf", name=f"x_bf{l + 1}")
            x_T_new = xpool.tile([P, DT, T], BF, tag="xT", name=f"x_T{l + 1}")
            xmid = mixpool.tile([P, TC, D], FP32, tag="xmid", name=f"xm{l}")
            last = l == N_LAYER - 1
            if last:
                lps = cpsum.tile([P, TC, T], FP32, tag="mm", name="lps")
                lout = mixpool.tile([P, TC, VOCAB], FP32, tag="lout", name="lout")
            ysum = mixpool.tile([P, TC, D], BF, tag="ysum", name=f"ys{l}")
            for hv in range(2):
                jps = list(range(hv * 2, hv * 2 + 2))
                nc.vector.tensor_add(
                    ysum[:, ds(hv * 2, 2), :],
                    ymr1[:, ds(hv * 2, 2), :],
                    ymr2[hv],
                )
                ystats = layer_norm_stats_batch(
                    [ysum[:, jp, :] for jp in jps], f"y{l}_{hv}"
                )
                for ji, jp in enumerate(jps):
                    nc.vector.scalar_tensor_tensor(
                        out=xmid[:, jp, :],
                        in0=ysum[:, jp, :],
                        scalar=ystats[ji][1],
                        in1=x_f[:, jp, :],
                        op0=ALU.mult,
                        op1=ALU.add,
                    )
                xstats = layer_norm_stats_batch(
                    [xmid[:, jp, :] for jp in jps], f"x{l}_{hv}"
                )
                for ji, jp in enumerate(jps):
                    mv2, r2 = xstats[ji]
                    nc.vector.tensor_scalar(
                        out=x_bf_new[:, jp, :],
                        in0=xmid[:, jp, :],
                        scalar1=mv2[:, 0:1],
                        scalar2=r2,
                        op0=ALU.subtract,
                        op1=ALU.mult,
                    )
                # One PSUM tile per chunk-pair for all four transposes —
                # halves the quad-ring pressure so the next layer's split-A
                # quads are not WAR-blocked behind the hv=1 transposes.
                tp = apsum.tile(
                    [P, DT, 2, P], BF, tag="quad", name=f"xtp{l}_{hv}"
                )
                for dt_ in range(DT):
                    for ji, jp in enumerate(jps):
                        nc.tensor.transpose(
                            tp[:, dt_, ji, :],
                            x_bf_new[:, jp, ds(dt_ * P, P)],
                            ident,
                        )
                for dt_ in range(DT):
                    nc.scalar.copy(
                        out=x_T_new[:, dt_, ds(hv * HT, HT)].rearrange(
                            "p (a b) -> p a b", a=2
                        ),
                        in_=tp[:, dt_],
                    )
                if last:
                    # lm head folded into the final boundary, per chunk-pair.
                    for jp in jps:
                        for dt_ in range(DT):
                            nc.tensor.matmul(
                                lps[:, jp, :VOCAB],
                                lhsT=x_T_new[:, dt_, ds(jp * P, P)],
                                rhs=lmt[:, dt_, :],
                                start=(dt_ == 0),
                                stop=(dt_ == DT - 1),
                            )
                    nc.scalar.copy(
                        out=lout[:, ds(hv * 2, 2), :],
                        in_=lps[:, ds(hv * 2, 2), :VOCAB],
                    )
                    nc.sync.dma_start(
                        out=out_d[:, ds(hv * 2, 2), :],
                        in_=lout[:, ds(hv * 2, 2), :],
                    )
                else:
                    for ji, jp in enumerate(jps):
                        mv2, r2 = xstats[ji]
                        nc.vector.tensor_scalar(
                            out=x_f_new[:, jp, :],
                            in0=xmid[:, jp, :],
                            scalar1=mv2[:, 0:1],
                            scalar2=r2,
                            op0=ALU.subtract,
                            op1=ALU.mult,
                        )
            return x_f_new, x_bf_new, x_T_new

        for l in range(N_LAYER):
            x_f, x_bf, x_T = emit_layer(l, x_f, x_bf, x_T)

    if not nc.is_finalized():
        nc.finalize()
    return nc


def _ln_np(x):
    m = x.mean(-1, keepdims=True)
    v = ((x - m) ** 2).mean(-1, keepdims=True)
    return (x - m) / np.sqrt(v + EPS)


def _make_tables():
    t = np.arange(N, dtype=np.float32)
    q = np.floor(t / 2.0) * 2.0
    freqs = (1.0 / (THETA ** (q / N)) / (2.0 * np.float32(math.pi))).astype(
        np.float32
    )
    phases = np.arange(T, dtype=np.float32)[:, None] * freqs[None, :]
    ph = np.float32(np.float32(phases % 1.0) * np.float32(2.0 * math.pi))
    return np.cos(ph).astype(np.float32), np.sin(ph).astype(np.float32)


def _prep_inputs(idx, embed_w, encoder, encoder_v, decoder, lm_head):
    perm = np.concatenate([np.arange(HALF) * 2, np.arange(HALF) * 2 + 1])

    cos, sin = _make_tables()
    cosp = cos[:, perm]
    sinp = sin[:, perm].copy()
    sinp[:, :HALF] *= -1.0
    # [P, NT, T]: (p, nt, t) -> table[t, nt*P + p]
    cos_h = np.ascontiguousarray(
        cosp.T.reshape(NT, P, T).transpose(1, 0, 2)
    ).astype(BF16)
    sin_h = np.ascontiguousarray(
        sinp.T.reshape(NT, P, T).transpose(1, 0, 2)
    ).astype(BF16)

    mask_h = np.zeros((P, TC, T), np.float32)
    t_idx = np.arange(T)
    for j in range(TC):
        for p in range(P):
            mask_h[p, j] = (t_idx > (j * P + p)).astype(np.float32)
    mask_h = mask_h.astype(BF16)

    lm_h = np.ascontiguousarray(
        lm_head.reshape(DT, P, VOCAB).transpose(1, 0, 2)
    ).astype(BF16)

    x0 = _ln_np(embed_w[idx].astype(np.float32))  # (B, T, D)

    dec3 = decoder.reshape(NH, N, D)

    per_core = []
    for core in range(N_CORES):
        b, h = divmod(core, NH)
        enc_p = encoder[h][:, perm]  # (D, N)
        encv_p = encoder_v[h][:, perm]
        dec_p = dec3[h][perm, :]  # (N, D)

        enc_h = enc_p.reshape(DT, P, NT, P).astype(BF16)
        encv_h = encv_p.reshape(DT, P, NT, P).astype(BF16)
        dec_h = np.ascontiguousarray(
            dec_p.reshape(NT, P, D).transpose(1, 0, 2)
        ).astype(BF16)

        xb = x0[b]  # (T, D) f32
        x0_c = np.ascontiguousarray(
            xb.reshape(TC, P, D).transpose(1, 0, 2)
        ).astype(np.float32)
        x0bf_c = x0_c.astype(BF16)
        x0T_c = np.ascontiguousarray(
            xb.T.reshape(DT, P, T).transpose(1, 0, 2)
        ).astype(BF16)

        per_core.append(
            {
                "x0": x0_c,
                "x0bf": x0bf_c,
                "x0T": x0T_c,
                "enc": enc_h,
                "encv": encv_h,
                "dec": dec_h,
                "cosb": cos_h,
                "sinb": sin_h,
                "maskb": mask_h,
                "lm": lm_h,
            }
        )
    return per_core


def _get_nc():
    if "nc" not in _CACHE:
        _CACHE["nc"] = _build_bass()
    return _CACHE["nc"]


def kernel(idx, embed_w, encoder, encoder_v, decoder, lm_head, **extra):
    idx = np.asarray(idx)
    embed_w = np.asarray(embed_w, dtype=np.float32)
    encoder = np.asarray(encoder, dtype=np.float32)
    encoder_v = np.asarray(encoder_v, dtype=np.float32)
    decoder = np.asarray(decoder, dtype=np.float32)
    lm_head = np.asarray(lm_head, dtype=np.float32)

    nc = _get_nc()
    in_maps = _prep_inputs(idx, embed_w, encoder, encoder_v, decoder, lm_head)
    res = run_bass_kernel_spmd(nc, in_maps, core_ids=list(range(N_CORES)))
    _CACHE["last_results"] = res

    out = np.zeros((B, T, VOCAB), np.float32)
    for b in range(B):
        lg = res.results[b * NH]["logits"]  # [P, TC, VOCAB]
        out[b] = lg.transpose(1, 0, 2).reshape(T, VOCAB)
    return out


if __name__ == "__main__":
    rng = np.random.default_rng(0)
    ins = {
        "idx": rng.integers(0, VOCAB, (B, T)).astype(np.int32),
        "embed_w": (0.02 * rng.standard_normal((VOCAB, D))).astype(np.float32),
        "encoder": (0.02 * rng.standard_normal((NH, D, N))).astype(np.float32),
        "encoder_v": (0.02 * rng.standard_normal((NH, D, N))).astype(np.float32),
        "decoder": (0.02 * rng.standard_normal((NH * N, D))).astype(np.float32),
        "lm_head": (0.02 * rng.standard_normal((D, VOCAB))).astype(np.float32),
    }
    out = kernel(**ins)
    print("out", out.shape, out.dtype, float(np.abs(out).max()))
